# revision 79
# baseline (speedup 1.0000x reference)
"""Trainium2 Bass kernel for nn_AccessControlHead (segment_reduce).

Reference computation: per-graph means of node_features columns 11 and 24
(segment-sum over a sorted batch index, G=2048 graphs), then a tiny MLP
  score = sigmoid(relu((1-means) @ W1.T + b1) @ W2.T + b2), 0 for empty graphs.

Strategy: batch is sorted, so each graph's nodes are contiguous. We shard
graph-aligned: 8 cores x 128 partitions x 2 graphs per partition = 2048
graphs. The host packs, for every partition, the node values of its two
graphs (padded to a common length R) with the graph slot encoded in the
SIGN: slot 0 nodes keep v, slot 1 nodes carry -v (v > 0 is guaranteed by
a host-side nudge of exact zeros). Each core recovers, per partition:
  counts  c0 = #(vs0 > 0), c1 = #(vs0 < 0)   (DVE compares, fused accum)
  sums    s0 = sum(relu(vs)), s1 = sum(relu(-vs))
with the six reductions expressed as accumulating tensor_scalar /
activation ops: col-11 sums run on ScalarE (hidden under the second half
of the input DMA) while DVE does the counts; DVE does the col-24 sums.
The tiny MLP runs per slot with scalar_tensor_tensor chains (hidden dim
on the free axis) and one ScalarE sigmoid. No collectives: every core
fully owns 256 consecutive graphs. Empty-graph masking adds -1e30 to the
sigmoid argument (sigmoid(-1e30) == 0.0).

Raw-bass notes (hard-won): engine ops do NOT observe the immediately
preceding op's SBUF writes -> explicit drain() between dependent ops;
PE/DVE reject partition offsets not in {0,32,64}; 3-D tensor_reduce
mis-reduces on HW; plain tensor_reduce runs ~4x slower than the 2x_2p
accumulating tensor_scalar form.
"""

import os
from contextlib import ExitStack

import numpy as np

import concourse.bass as bass
from concourse import mybir
from concourse.bass_utils import run_bass_kernel_spmd

G = 2048
N_CORES = 8
P = 128  # partitions per core
H = 32  # MLP hidden dim
MODIFIER_COL = 11
OWNER_COL = 24
NPAR = 163  # params tail: W1c0 | W1c1 | -W1c1 | b1 | [W2, -1, b2]

F32 = mybir.dt.float32

# populated when BASS_KERNEL_PROFILE=1 so a harness can report HW time
LAST_EXEC_TIME_NS = None
LAST_PROFILE = None
LAST_R = 1088


def _bcast(ap: bass.AP, axis: int, n: int) -> bass.AP:
    """Insert a stride-0 broadcast dim of size n at position `axis` of an AP."""
    pat = list(ap.ap)
    pat.insert(axis, [0, n])
    return bass.AP(tensor=ap.tensor, offset=ap.offset, ap=pat)


def _lean_bass() -> bass.Bass:
    """Bass() without the const-AP memsets + all-engine barrier preamble
    (~0.7us). The kernel supplies its own zero bias tile for activations,
    so the const APs are never read."""
    orig_memset = bass.BassSharedVectorInterface.memset
    orig_barrier = bass.Bass.all_engine_barrier
    bass.BassSharedVectorInterface.memset = lambda self, ap, c: None
    bass.Bass.all_engine_barrier = lambda self, *a, **k: None
    try:
        return bass.Bass()
    finally:
        bass.BassSharedVectorInterface.memset = orig_memset
        bass.Bass.all_engine_barrier = orig_barrier


def _build_bass(R: int) -> bass.Bass:
    nc = _lean_bass()

    # input blob per partition row:
    #   [:, 0:R]        signed node values col 11 (vs0), zero-padded
    #   [:, R:2R]       signed node values col 24 (vs1)
    #   [:, 2R+0:+32]    W1[:,0] (replicated on every partition)
    #   [:, 2R+32:+64]   W1[:,1]
    #   [:, 2R+64:+96]   -W1[:,1] (for the w=1 chain: its af_c1 is negated)
    #   [:, 2R+96:+128]  b1
    #   [:, 2R+128:+160] W2[0,:]
    #   [:, 2R+160:+162] [-1, b2] (extra W2 coefficients for the penalty
    #                    and bias hidden units)
    blob = nc.dram_tensor("blob", [P, 2 * R + NPAR], F32, kind="ExternalInput")
    out = nc.dram_tensor("score_out", [P, 2], F32, kind="ExternalOutput")

    AX = mybir.AxisListType.X
    OP = mybir.AluOpType
    ACT = mybir.ActivationFunctionType
    q = 2 * R

    with ExitStack() as ctx:

        def sb(name, shape):
            return ctx.enter_context(nc.sbuf_tensor(name, shape, F32))

        block = ctx.enter_context(nc.Block())
        dma_sem = ctx.enter_context(nc.semaphore("dma_sem"))
        dve_sem = ctx.enter_context(nc.semaphore("dve_sem"))
        act_sem = ctx.enter_context(nc.semaphore("act_sem"))

        sb_blob = sb("sb_blob", [P, 2 * R + NPAR])
        junk0 = sb("junk0", [P, R])
        junkA = sb("junkA", [P, R])
        cnt2 = sb("cnt2", [P, 2])  # [c0, c1]
        sab = sb("sab", [P, 4])  # [s0_c0, s1_c0, s0_c1, s1_c1]
        ones2 = sb("ones2", [P, 2])
        den = sb("den", [P, 2])
        rec = sb("rec", [P, 2])
        af = sb("af", [P, 4])  # [af_c0w0, af_c0w1, af_c1w0, af_c1w1]
        t0 = sb("t0", [P, 2, H])
        t1 = sb("t1", [P, 2, H + 2])
        hws = sb("hws", [P, 2, H + 2])
        zs = sb("zs", [P, 2])
        pen = sb("pen", [P, 2])
        z = sb("z", [P, 2])
        sc = sb("sc", [P, 2])
        zt = sb("zt", [P, 1])  # ACT-owned zero bias (replaces const APs)

        vs0 = sb_blob[:, 0:R]
        vs1 = sb_blob[:, R : 2 * R]

        @block.sync
        def _(sync):
            # split the input DMA: col-11 region first so DVE starts early
            sync.dma_start(out=sb_blob[:, 0:R], in_=blob[:, 0:R]).then_inc(
                dma_sem, 16
            )
            sync.dma_start(
                out=sb_blob[:, R : 2 * R + NPAR], in_=blob[:, R : 2 * R + NPAR]
            ).then_inc(dma_sem, 16)
            sync.wait_ge(dma_sem, 48)  # in x2 + act-issued out

        @block.scalar
        def _(act):
            act.memzero(zt[:])
            act.drain()
            # col-11 per-slot sums via Relu accumulation, hidden under the
            # second DMA: s0 = sum(relu(vs0)), s1 = sum(relu(-vs0))
            act.wait_ge(dma_sem, 16)
            act.activation(
                out=junkA[:], in_=vs0, func=ACT.Relu, bias=zt[:], accum_out=sab[:, 0:1]
            )
            act.activation(
                out=junkA[:],
                in_=vs0,
                func=ACT.Relu,
                bias=zt[:],
                scale=-1.0,
                accum_out=sab[:, 1:2],
            ).then_inc(act_sem, 1)
            # final sigmoid after the DVE tail, then write scores out
            # (drain so the DMA engine cannot read sc before sigmoid lands)
            act.wait_ge(dve_sem, 1)
            act.activation(out=sc[:], in_=z[:], func=ACT.Sigmoid, bias=zt[:])
            act.drain()
            act.dma_start(out=out[:], in_=sc[:]).then_inc(dma_sem, 16)

        @block.vector
        def _(dve):
            # raw-bass engine ops do NOT see the immediately preceding op's
            # SBUF writes; drain between dependent ops
            D = dve.drain
            dve.memset(ones2[:], 1.0)
            dve.memset(t1[:, 0, H + 1 : H + 2], 1.0)
            dve.memset(t1[:, 1, H + 1 : H + 2], 1.0)
            dve.wait_ge(dma_sem, 16)

            # reductions as accumulating tensor_scalar (2x_2p DVE perf mode;
            # accum variant: out = in op0 s1, accum_out = reduce(out, op1));
            # counts from the sign of vs0
            dve.tensor_scalar(
                junk0[:], vs0, 0.0, None, OP.is_gt, OP.add, accum_out=cnt2[:, 0:1]
            )
            dve.tensor_scalar(
                junk0[:], vs0, 0.0, None, OP.is_lt, OP.add, accum_out=cnt2[:, 1:2]
            )
            D()
            # den = max(c,1)*-1 -> rec = -1/max(c,1); pen for empty graphs
            # (these run while the second DMA is still in flight)
            dve.tensor_scalar(den[:], cnt2[:], 1.0, -1.0, OP.max, OP.mult)
            dve.tensor_scalar(pen[:], cnt2[:], 0.0, 1e30, OP.is_le, OP.mult)
            D()
            dve.reciprocal(rec[:], den[:])
            # col-24 sums once the second DMA lands: s0 = sum(max(vs1,0)),
            # s1 = -sum(min(vs1,0)) -- negation handled via the subtract-form
            # af and the host-negated W1[:,1] copy
            dve.wait_ge(dma_sem, 32)
            dve.tensor_scalar(
                junk0[:], vs1, 0.0, None, OP.max, OP.add, accum_out=sab[:, 2:3]
            )
            dve.tensor_scalar(
                junk0[:], vs1, 0.0, None, OP.min, OP.add, accum_out=sab[:, 3:4]
            )
            dve.wait_ge(act_sem, 1)
            D()
            # access features as per-partition scalars, one level:
            #   af[:,0] = af_c0w0 = s0c0*rec0 + 1
            #   af[:,1] = af_c0w1 = s1c0*rec1 + 1
            #   af[:,2] = af_c1w0 = s0c1*rec0 + 1
            #   af[:,3] = -af_c1w1 = (-s1c1)*rec1 - 1   (sab3 holds -s1c1;
            #            compensated by the host-negated W1[:,1] copy below)
            one1 = ones2[:, 0:1]
            dve.scalar_tensor_tensor(
                af[:, 0:3:2], sab[:, 0:3:2], rec[:, 0:1], ones2[:], OP.mult, OP.add
            )
            dve.scalar_tensor_tensor(
                af[:, 1:2], sab[:, 1:2], rec[:, 1:2], one1, OP.mult, OP.add
            )
            dve.scalar_tensor_tensor(
                af[:, 3:4], sab[:, 3:4], rec[:, 1:2], one1, OP.mult, OP.subtract
            )
            D()
            # hidden layer per slot w, hidden dim along the free axis:
            #   v_w = af_c1w*W1[:,1] + b1 ; u_w = af_c0w*W1[:,0] + v_w
            w1c0r = sb_blob[:, q : q + H]
            w1c1r = sb_blob[:, q + H : q + 2 * H]
            w1c1n = sb_blob[:, q + 2 * H : q + 3 * H]  # -W1[:,1]
            b1r = sb_blob[:, q + 3 * H : q + 4 * H]
            w2r = sb_blob[:, q + 4 * H : q + 5 * H]
            dve.scalar_tensor_tensor(
                t0[:, 0, :], w1c1r, af[:, 2:3], b1r, OP.mult, OP.add
            )
            dve.scalar_tensor_tensor(
                t0[:, 1, :], w1c1n, af[:, 3:4], b1r, OP.mult, OP.add
            )
            D()
            dve.scalar_tensor_tensor(
                t1[:, 0, 0:H], w1c0r, af[:, 0:1], t0[:, 0, :], OP.mult, OP.add
            )
            dve.scalar_tensor_tensor(
                t1[:, 1, 0:H], w1c0r, af[:, 1:2], t0[:, 1, :], OP.mult, OP.add
            )
            # two extra hidden units ride along: u[32] = +1e30*empty (W2
            # coefficient -1 -> the empty-graph penalty) and u[33] = 1
            # (W2 coefficient b2 -> the bias); both survive the relu
            dve.tensor_copy(t1[:, 0, H : H + 1], pen[:, 0:1])
            dve.tensor_copy(t1[:, 1, H : H + 1], pen[:, 1:2])
            D()
            # z = sum_j relu(u_j) * w2ext_j  (w2ext = [W2, -1, b2])
            w2e = sb_blob[:, q + 4 * H : q + 5 * H + 2]
            dve.scalar_tensor_tensor(
                hws[:, 0, :], t1[:, 0, :], 0.0, w2e, OP.max, OP.mult,
                accum_out=z[:, 0:1],
            )
            dve.scalar_tensor_tensor(
                hws[:, 1, :], t1[:, 1, :], 0.0, w2e, OP.max, OP.mult,
                accum_out=z[:, 1:2],
            )
            D()
            dve.nop().then_inc(dve_sem, 1)

    return nc


def kernel(
    node_features,
    batch,
    graph_embedding=None,
    W1=None,
    b1=None,
    W2=None,
    b2=None,
    num_graphs=None,
    **_unused,
):
    global LAST_EXEC_TIME_NS, LAST_PROFILE

    node_features = np.asarray(node_features)
    batch = np.asarray(batch)
    N = batch.shape[0]

    # CSR-style boundaries of the sorted index (pure index prep, O(G log N))
    bounds = np.searchsorted(batch, np.arange(G + 1, dtype=batch.dtype))
    pb = bounds[0::2]  # starts of even graphs + total end: 1025 entries
    pair_counts = np.diff(pb)
    R = max(1088, int(np.ceil(pair_counts.max() / 64.0)) * 64)

    # signed values: slot-1 (odd graph) nodes negated; exact zeros nudged so
    # the device can recover counts from the sign (values are >= 0 here)
    sgn = 1.0 - 2.0 * (batch & 1).astype(np.float32)
    colsT = np.empty((2, N), np.float32)
    colsT[0] = node_features[:, MODIFIER_COL]
    colsT[1] = node_features[:, OWNER_COL]
    assert colsT[0].min() >= 0.0, "sign encoding needs non-negative col values"
    zz = colsT[0] == 0.0
    if zz.any():
        colsT[0, zz] = 1e-30
    vsT = colsT * sgn

    W1 = np.asarray(W1, np.float32)  # [32, 2]
    W2 = np.asarray(W2, np.float32)  # [1, 32]
    b1 = np.asarray(b1, np.float32)
    b2v = np.float32(np.asarray(b2, np.float32).reshape(()))

    blob_all = np.zeros((N_CORES, P, 2 * R + NPAR), np.float32)
    q = 2 * R
    blob_all[:, :, q + 0 * H : q + 1 * H] = W1[:, 0]
    blob_all[:, :, q + 1 * H : q + 2 * H] = W1[:, 1]
    blob_all[:, :, q + 2 * H : q + 3 * H] = -W1[:, 1]
    blob_all[:, :, q + 3 * H : q + 4 * H] = b1
    blob_all[:, :, q + 4 * H : q + 5 * H] = W2[0]
    blob_all[:, :, q + 5 * H] = -1.0
    blob_all[:, :, q + 5 * H + 1] = b2v
    flat = blob_all.reshape(N_CORES * P, 2 * R + NPAR)
    for i in range(N_CORES * P):
        s, t = pb[i], pb[i + 1]
        if t > s:
            L = t - s
            flat[i, 0:L] = vsT[0, s:t]
            flat[i, R : R + L] = vsT[1, s:t]

    in_maps = [{"blob": blob_all[c]} for c in range(N_CORES)]

    global LAST_R
    LAST_R = R
    nc = _build_bass(R)
    trace = os.environ.get("BASS_KERNEL_PROFILE") == "1"
    res = run_bass_kernel_spmd(nc, in_maps, list(range(N_CORES)), trace=trace)
    LAST_EXEC_TIME_NS = getattr(res, "exec_time_ns", None)
    LAST_PROFILE = getattr(res, "profile_json", None)

    # partition p of core c holds graphs 256c + 2p (w=0) and 256c + 2p+1 (w=1)
    scores = np.empty((G,), np.float32)
    for c in range(N_CORES):
        scores[c * 2 * P : (c + 1) * 2 * P] = res.results[c]["score_out"].ravel()
    return scores


# revision 82
# speedup vs baseline: 1.0368x; 1.0368x over previous
"""Trainium2 Bass kernel for nn_AccessControlHead (segment_reduce).

Reference computation: per-graph means of node_features columns 11 and 24
(segment-sum over a sorted batch index, G=2048 graphs), then a tiny MLP
  score = sigmoid(relu((1-means) @ W1.T + b1) @ W2.T + b2), 0 for empty graphs.

Strategy: batch is sorted, so each graph's nodes are contiguous. We shard
graph-aligned: 8 cores x 128 partitions x 2 graphs per partition = 2048
graphs. Graphs are paired largest-with-smallest by node count so the
padded row length R tracks 2x the mean count instead of the worst pair.
The host packs, for every partition, the node values of its two graphs
(padded to a common length R) with the graph slot encoded in the SIGN:
slot 0 nodes keep v, slot 1 nodes carry -v (v > 0 is guaranteed by a
host-side nudge of exact zeros). Each core recovers, per partition:
  counts  c0 = #(vs0 > 0), c1 = #(vs0 < 0)   (DVE compares, fused accum)
  sums    s0 = sum(relu(vs)), s1 = sum(relu(-vs))
with the six reductions expressed as accumulating tensor_scalar /
activation ops: col-11 sums run on ScalarE (hidden under the second half
of the input DMA) while DVE does the counts; DVE does the col-24 sums.
The tiny MLP runs per slot with scalar_tensor_tensor chains (hidden dim
on the free axis) and one ScalarE sigmoid. No collectives: every core
fully owns 256 consecutive graphs. Empty-graph masking adds -1e30 to the
sigmoid argument (sigmoid(-1e30) == 0.0).

Raw-bass notes (hard-won): engine ops do NOT observe the immediately
preceding op's SBUF writes -> explicit drain() between dependent ops;
PE/DVE reject partition offsets not in {0,32,64}; 3-D tensor_reduce
mis-reduces on HW; plain tensor_reduce runs ~4x slower than the 2x_2p
accumulating tensor_scalar form.
"""

import os
from contextlib import ExitStack

import numpy as np

import concourse.bass as bass
from concourse import mybir
from concourse.bass_utils import run_bass_kernel_spmd

G = 2048
N_CORES = 8
P = 128  # partitions per core
H = 32  # MLP hidden dim
MODIFIER_COL = 11
OWNER_COL = 24
NPAR = 163  # params tail: W1c0 | W1c1 | -W1c1 | b1 | [W2, -1, b2]

F32 = mybir.dt.float32

# populated when BASS_KERNEL_PROFILE=1 so a harness can report HW time
LAST_EXEC_TIME_NS = None
LAST_PROFILE = None
LAST_R = 1088


def _bcast(ap: bass.AP, axis: int, n: int) -> bass.AP:
    """Insert a stride-0 broadcast dim of size n at position `axis` of an AP."""
    pat = list(ap.ap)
    pat.insert(axis, [0, n])
    return bass.AP(tensor=ap.tensor, offset=ap.offset, ap=pat)


def _lean_bass() -> bass.Bass:
    """Bass() without the const-AP memsets + all-engine barrier preamble
    (~0.7us). The kernel supplies its own zero bias tile for activations,
    so the const APs are never read."""
    orig_memset = bass.BassSharedVectorInterface.memset
    orig_barrier = bass.Bass.all_engine_barrier
    bass.BassSharedVectorInterface.memset = lambda self, ap, c: None
    bass.Bass.all_engine_barrier = lambda self, *a, **k: None
    try:
        return bass.Bass()
    finally:
        bass.BassSharedVectorInterface.memset = orig_memset
        bass.Bass.all_engine_barrier = orig_barrier


def _build_bass(R: int) -> bass.Bass:
    nc = _lean_bass()

    # input blob per partition row:
    #   [:, 0:R]        signed node values col 11 (vs0), zero-padded
    #   [:, R:2R]       signed node values col 24 (vs1)
    #   [:, 2R+0:+32]    W1[:,0] (replicated on every partition)
    #   [:, 2R+32:+64]   W1[:,1]
    #   [:, 2R+64:+96]   -W1[:,1] (for the w=1 chain: its af_c1 is negated)
    #   [:, 2R+96:+128]  b1
    #   [:, 2R+128:+160] W2[0,:]
    #   [:, 2R+160:+162] [-1, b2] (extra W2 coefficients for the penalty
    #                    and bias hidden units)
    blob = nc.dram_tensor("blob", [P, 2 * R + NPAR], F32, kind="ExternalInput")
    out = nc.dram_tensor("score_out", [P, 2], F32, kind="ExternalOutput")

    AX = mybir.AxisListType.X
    OP = mybir.AluOpType
    ACT = mybir.ActivationFunctionType
    q = 2 * R

    with ExitStack() as ctx:

        def sb(name, shape):
            return ctx.enter_context(nc.sbuf_tensor(name, shape, F32))

        block = ctx.enter_context(nc.Block())
        dma_sem = ctx.enter_context(nc.semaphore("dma_sem"))
        dve_sem = ctx.enter_context(nc.semaphore("dve_sem"))
        act_sem = ctx.enter_context(nc.semaphore("act_sem"))

        sb_blob = sb("sb_blob", [P, 2 * R + NPAR])
        junk0 = sb("junk0", [P, R])
        junkA = sb("junkA", [P, R])
        cnt2 = sb("cnt2", [P, 2])  # [c0, c1]
        sab = sb("sab", [P, 4])  # [s0_c0, s1_c0, s0_c1, s1_c1]
        ones2 = sb("ones2", [P, 2])
        den = sb("den", [P, 2])
        rec = sb("rec", [P, 2])
        af = sb("af", [P, 4])  # [af_c0w0, af_c0w1, af_c1w0, af_c1w1]
        t0 = sb("t0", [P, 2, H])
        t1 = sb("t1", [P, 2, H + 2])
        hws = sb("hws", [P, 2, H + 2])
        zs = sb("zs", [P, 2])
        pen = sb("pen", [P, 2])
        z = sb("z", [P, 2])
        sc = sb("sc", [P, 2])
        zt = sb("zt", [P, 1])  # ACT-owned zero bias (replaces const APs)

        vs0 = sb_blob[:, 0:R]
        vs1 = sb_blob[:, R : 2 * R]

        @block.sync
        def _(sync):
            # split the input DMA: col-11 region first so DVE starts early
            sync.dma_start(out=sb_blob[:, 0:R], in_=blob[:, 0:R]).then_inc(
                dma_sem, 16
            )
            sync.dma_start(
                out=sb_blob[:, R : 2 * R + NPAR], in_=blob[:, R : 2 * R + NPAR]
            ).then_inc(dma_sem, 16)
            sync.wait_ge(dma_sem, 48)  # in x2 + act-issued out

        @block.scalar
        def _(act):
            act.memzero(zt[:])
            act.drain()
            # col-11 per-slot sums via Relu accumulation, hidden under the
            # second DMA: s0 = sum(relu(vs0)), s1 = sum(relu(-vs0))
            act.wait_ge(dma_sem, 16)
            act.activation(
                out=junkA[:], in_=vs0, func=ACT.Relu, bias=zt[:], accum_out=sab[:, 0:1]
            )
            act.activation(
                out=junkA[:],
                in_=vs0,
                func=ACT.Relu,
                bias=zt[:],
                scale=-1.0,
                accum_out=sab[:, 1:2],
            ).then_inc(act_sem, 1)
            # final sigmoid after the DVE tail, then write scores out
            # (drain so the DMA engine cannot read sc before sigmoid lands)
            act.wait_ge(dve_sem, 1)
            act.activation(out=sc[:], in_=z[:], func=ACT.Sigmoid, bias=zt[:])
            act.drain()
            act.dma_start(out=out[:], in_=sc[:]).then_inc(dma_sem, 16)

        @block.vector
        def _(dve):
            # raw-bass engine ops do NOT see the immediately preceding op's
            # SBUF writes; drain between dependent ops
            D = dve.drain
            dve.memset(ones2[:], 1.0)
            dve.memset(t1[:, 0, H + 1 : H + 2], 1.0)
            dve.memset(t1[:, 1, H + 1 : H + 2], 1.0)
            dve.wait_ge(dma_sem, 16)

            # reductions as accumulating tensor_scalar (2x_2p DVE perf mode;
            # accum variant: out = in op0 s1, accum_out = reduce(out, op1));
            # counts from the sign of vs0
            dve.tensor_scalar(
                junk0[:], vs0, 0.0, None, OP.is_gt, OP.add, accum_out=cnt2[:, 0:1]
            )
            dve.tensor_scalar(
                junk0[:], vs0, 0.0, None, OP.is_lt, OP.add, accum_out=cnt2[:, 1:2]
            )
            D()
            # den = max(c,1)*-1 -> rec = -1/max(c,1); pen for empty graphs
            # (these run while the second DMA is still in flight)
            dve.tensor_scalar(den[:], cnt2[:], 1.0, -1.0, OP.max, OP.mult)
            dve.tensor_scalar(pen[:], cnt2[:], 0.0, 1e30, OP.is_le, OP.mult)
            D()
            dve.reciprocal(rec[:], den[:])
            # col-24 sums once the second DMA lands: s0 = sum(max(vs1,0)),
            # s1 = -sum(min(vs1,0)) -- negation handled via the subtract-form
            # af and the host-negated W1[:,1] copy
            dve.wait_ge(dma_sem, 32)
            dve.tensor_scalar(
                junk0[:], vs1, 0.0, None, OP.max, OP.add, accum_out=sab[:, 2:3]
            )
            dve.tensor_scalar(
                junk0[:], vs1, 0.0, None, OP.min, OP.add, accum_out=sab[:, 3:4]
            )
            dve.wait_ge(act_sem, 1)
            D()
            # access features as per-partition scalars, one level:
            #   af[:,0] = af_c0w0 = s0c0*rec0 + 1
            #   af[:,1] = af_c0w1 = s1c0*rec1 + 1
            #   af[:,2] = af_c1w0 = s0c1*rec0 + 1
            #   af[:,3] = -af_c1w1 = (-s1c1)*rec1 - 1   (sab3 holds -s1c1;
            #            compensated by the host-negated W1[:,1] copy below)
            one1 = ones2[:, 0:1]
            dve.scalar_tensor_tensor(
                af[:, 0:3:2], sab[:, 0:3:2], rec[:, 0:1], ones2[:], OP.mult, OP.add
            )
            dve.scalar_tensor_tensor(
                af[:, 1:2], sab[:, 1:2], rec[:, 1:2], one1, OP.mult, OP.add
            )
            dve.scalar_tensor_tensor(
                af[:, 3:4], sab[:, 3:4], rec[:, 1:2], one1, OP.mult, OP.subtract
            )
            D()
            # hidden layer per slot w, hidden dim along the free axis:
            #   v_w = af_c1w*W1[:,1] + b1 ; u_w = af_c0w*W1[:,0] + v_w
            w1c0r = sb_blob[:, q : q + H]
            w1c1r = sb_blob[:, q + H : q + 2 * H]
            w1c1n = sb_blob[:, q + 2 * H : q + 3 * H]  # -W1[:,1]
            b1r = sb_blob[:, q + 3 * H : q + 4 * H]
            w2r = sb_blob[:, q + 4 * H : q + 5 * H]
            dve.scalar_tensor_tensor(
                t0[:, 0, :], w1c1r, af[:, 2:3], b1r, OP.mult, OP.add
            )
            dve.scalar_tensor_tensor(
                t0[:, 1, :], w1c1n, af[:, 3:4], b1r, OP.mult, OP.add
            )
            D()
            dve.scalar_tensor_tensor(
                t1[:, 0, 0:H], w1c0r, af[:, 0:1], t0[:, 0, :], OP.mult, OP.add
            )
            dve.scalar_tensor_tensor(
                t1[:, 1, 0:H], w1c0r, af[:, 1:2], t0[:, 1, :], OP.mult, OP.add
            )
            # two extra hidden units ride along: u[32] = +1e30*empty (W2
            # coefficient -1 -> the empty-graph penalty) and u[33] = 1
            # (W2 coefficient b2 -> the bias); both survive the relu
            dve.tensor_copy(t1[:, 0, H : H + 1], pen[:, 0:1])
            dve.tensor_copy(t1[:, 1, H : H + 1], pen[:, 1:2])
            D()
            # z = sum_j relu(u_j) * w2ext_j  (w2ext = [W2, -1, b2])
            w2e = sb_blob[:, q + 4 * H : q + 5 * H + 2]
            dve.scalar_tensor_tensor(
                hws[:, 0, :], t1[:, 0, :], 0.0, w2e, OP.max, OP.mult,
                accum_out=z[:, 0:1],
            )
            dve.scalar_tensor_tensor(
                hws[:, 1, :], t1[:, 1, :], 0.0, w2e, OP.max, OP.mult,
                accum_out=z[:, 1:2],
            )
            D()
            dve.nop().then_inc(dve_sem, 1)

    return nc


def kernel(
    node_features,
    batch,
    graph_embedding=None,
    W1=None,
    b1=None,
    W2=None,
    b2=None,
    num_graphs=None,
    **_unused,
):
    global LAST_EXEC_TIME_NS, LAST_PROFILE

    node_features = np.asarray(node_features)
    batch = np.asarray(batch)
    N = batch.shape[0]

    # CSR-style boundaries of the sorted index (pure index prep, O(G log N))
    bounds = np.searchsorted(batch, np.arange(G + 1, dtype=batch.dtype))
    counts = np.diff(bounds)
    # pair the largest-count graph with the smallest etc. so the padded
    # row length R tracks 2*mean instead of the worst adjacent pair
    order = np.argsort(counts, kind="stable")
    ga = order[: G // 2]  # slot-0 graph of pair j
    gb = order[G // 2 :][::-1]  # slot-1 graph of pair j
    R = int(np.ceil((counts[ga] + counts[gb]).max() / 8.0)) * 8

    # signed values: slot-1 nodes negated; exact zeros nudged so the
    # device can recover counts from the sign (values are >= 0 here)
    slot_of_graph = np.empty(G, np.int8)
    slot_of_graph[ga] = 0
    slot_of_graph[gb] = 1
    sgn = 1.0 - 2.0 * slot_of_graph[batch].astype(np.float32)
    colsT = np.empty((2, N), np.float32)
    colsT[0] = node_features[:, MODIFIER_COL]
    colsT[1] = node_features[:, OWNER_COL]
    assert colsT[0].min() >= 0.0, "sign encoding needs non-negative col values"
    zz = colsT[0] == 0.0
    if zz.any():
        colsT[0, zz] = 1e-30
    vsT = colsT * sgn

    W1 = np.asarray(W1, np.float32)  # [32, 2]
    W2 = np.asarray(W2, np.float32)  # [1, 32]
    b1 = np.asarray(b1, np.float32)
    b2v = np.float32(np.asarray(b2, np.float32).reshape(()))

    blob_all = np.zeros((N_CORES, P, 2 * R + NPAR), np.float32)
    q = 2 * R
    blob_all[:, :, q + 0 * H : q + 1 * H] = W1[:, 0]
    blob_all[:, :, q + 1 * H : q + 2 * H] = W1[:, 1]
    blob_all[:, :, q + 2 * H : q + 3 * H] = -W1[:, 1]
    blob_all[:, :, q + 3 * H : q + 4 * H] = b1
    blob_all[:, :, q + 4 * H : q + 5 * H] = W2[0]
    blob_all[:, :, q + 5 * H] = -1.0
    blob_all[:, :, q + 5 * H + 1] = b2v
    flat = blob_all.reshape(N_CORES * P, 2 * R + NPAR)
    for j in range(N_CORES * P):
        o = 0
        for g in (ga[j], gb[j]):
            s, t = bounds[g], bounds[g + 1]
            if t > s:
                L = t - s
                flat[j, o : o + L] = vsT[0, s:t]
                flat[j, R + o : R + o + L] = vsT[1, s:t]
                o += L

    in_maps = [{"blob": blob_all[c]} for c in range(N_CORES)]

    global LAST_R
    LAST_R = R
    nc = _build_bass(R)
    trace = os.environ.get("BASS_KERNEL_PROFILE") == "1"
    res = run_bass_kernel_spmd(nc, in_maps, list(range(N_CORES)), trace=trace)
    LAST_EXEC_TIME_NS = getattr(res, "exec_time_ns", None)
    LAST_PROFILE = getattr(res, "profile_json", None)

    # pair j = (core j//P, partition j%P); slot 0 -> graph ga[j], slot 1 -> gb[j]
    allres = np.concatenate(
        [res.results[c]["score_out"] for c in range(N_CORES)], axis=0
    )  # [1024, 2]
    scores = np.empty((G,), np.float32)
    scores[ga] = allres[:, 0]
    scores[gb] = allres[:, 1]
    return scores


# revision 84
# speedup vs baseline: 1.0379x; 1.0010x over previous
"""Trainium2 Bass kernel for nn_AccessControlHead (segment_reduce).

Reference computation: per-graph means of node_features columns 11 and 24
(segment-sum over a sorted batch index, G=2048 graphs), then a tiny MLP
  score = sigmoid(relu((1-means) @ W1.T + b1) @ W2.T + b2), 0 for empty graphs.

Strategy: batch is sorted, so each graph's nodes are contiguous. We shard
graph-aligned: 8 cores x 128 partitions x 2 graphs per partition = 2048
graphs. Graphs are paired largest-with-smallest by node count so the
padded row length R tracks 2x the mean count instead of the worst pair.
The host packs, for every partition, the node values of its two graphs
(padded to a common length R) with the graph slot encoded in the SIGN:
slot 0 nodes keep v, slot 1 nodes carry -v (v > 0 is guaranteed by a
host-side nudge of exact zeros). Each core recovers, per partition:
  counts  c0 = #(vs0 > 0), c1 = #(vs0 < 0)   (DVE compares, fused accum)
  sums    s0 = sum(relu(vs)), s1 = sum(relu(-vs))
with the six reductions expressed as accumulating tensor_scalar /
activation ops: col-11 sums run on ScalarE (hidden under the second half
of the input DMA) while DVE does the counts; DVE does the col-24 sums.
The tiny MLP runs per slot with scalar_tensor_tensor chains (hidden dim
on the free axis) and one ScalarE sigmoid. No collectives: every core
fully owns 256 consecutive graphs. Empty-graph masking adds -1e30 to the
sigmoid argument (sigmoid(-1e30) == 0.0).

Raw-bass notes (hard-won): engine ops do NOT observe the immediately
preceding op's SBUF writes -> explicit drain() between dependent ops;
PE/DVE reject partition offsets not in {0,32,64}; 3-D tensor_reduce
mis-reduces on HW; plain tensor_reduce runs ~4x slower than the 2x_2p
accumulating tensor_scalar form.
"""

import os
from concurrent.futures import ThreadPoolExecutor
from contextlib import ExitStack

import numpy as np

import concourse.bass as bass
from concourse import mybir
from concourse.bass_utils import run_bass_kernel_spmd

G = 2048
N_CORES = 8
P = 128  # partitions per core
H = 32  # MLP hidden dim
MODIFIER_COL = 11
OWNER_COL = 24
NPAR = 163  # params tail: W1c0 | W1c1 | -W1c1 | b1 | [W2, -1, b2]

F32 = mybir.dt.float32

# populated when BASS_KERNEL_PROFILE=1 so a harness can report HW time
LAST_EXEC_TIME_NS = None
LAST_PROFILE = None
LAST_R = 1088


def _bcast(ap: bass.AP, axis: int, n: int) -> bass.AP:
    """Insert a stride-0 broadcast dim of size n at position `axis` of an AP."""
    pat = list(ap.ap)
    pat.insert(axis, [0, n])
    return bass.AP(tensor=ap.tensor, offset=ap.offset, ap=pat)


def _lean_bass() -> bass.Bass:
    """Bass() without the const-AP memsets + all-engine barrier preamble
    (~0.7us). The kernel supplies its own zero bias tile for activations,
    so the const APs are never read."""
    orig_memset = bass.BassSharedVectorInterface.memset
    orig_barrier = bass.Bass.all_engine_barrier
    bass.BassSharedVectorInterface.memset = lambda self, ap, c: None
    bass.Bass.all_engine_barrier = lambda self, *a, **k: None
    try:
        return bass.Bass()
    finally:
        bass.BassSharedVectorInterface.memset = orig_memset
        bass.Bass.all_engine_barrier = orig_barrier


def _build_bass(R: int) -> bass.Bass:
    nc = _lean_bass()

    # input blob per partition row:
    #   [:, 0:R]        signed node values col 11 (vs0), zero-padded
    #   [:, R:2R]       signed node values col 24 (vs1)
    #   [:, 2R+0:+32]    W1[:,0] (replicated on every partition)
    #   [:, 2R+32:+64]   W1[:,1]
    #   [:, 2R+64:+96]   -W1[:,1] (for the w=1 chain: its af_c1 is negated)
    #   [:, 2R+96:+128]  b1
    #   [:, 2R+128:+160] W2[0,:]
    #   [:, 2R+160:+162] [-1, b2] (extra W2 coefficients for the penalty
    #                    and bias hidden units)
    blob = nc.dram_tensor("blob", [P, 2 * R + NPAR], F32, kind="ExternalInput")
    out = nc.dram_tensor("score_out", [P, 2], F32, kind="ExternalOutput")

    AX = mybir.AxisListType.X
    OP = mybir.AluOpType
    ACT = mybir.ActivationFunctionType
    q = 2 * R

    with ExitStack() as ctx:

        def sb(name, shape):
            return ctx.enter_context(nc.sbuf_tensor(name, shape, F32))

        block = ctx.enter_context(nc.Block())
        dma_sem = ctx.enter_context(nc.semaphore("dma_sem"))
        dve_sem = ctx.enter_context(nc.semaphore("dve_sem"))
        act_sem = ctx.enter_context(nc.semaphore("act_sem"))

        sb_blob = sb("sb_blob", [P, 2 * R + NPAR])
        junk0 = sb("junk0", [P, R])
        junkA = sb("junkA", [P, R])
        cnt2 = sb("cnt2", [P, 2])  # [c0, c1]
        sab = sb("sab", [P, 4])  # [s0_c0, s1_c0, s0_c1, s1_c1]
        ones2 = sb("ones2", [P, 2])
        den = sb("den", [P, 2])
        rec = sb("rec", [P, 2])
        af = sb("af", [P, 4])  # [af_c0w0, af_c0w1, af_c1w0, af_c1w1]
        t0 = sb("t0", [P, 2, H])
        t1 = sb("t1", [P, 2, H + 2])
        hws = sb("hws", [P, 2, H + 2])
        zs = sb("zs", [P, 2])
        pen = sb("pen", [P, 2])
        z = sb("z", [P, 2])
        sc = sb("sc", [P, 2])
        zt = sb("zt", [P, 1])  # ACT-owned zero bias (replaces const APs)

        vs0 = sb_blob[:, 0:R]
        vs1 = sb_blob[:, R : 2 * R]

        @block.sync
        def _(sync):
            # split the input DMA: col-11 region first so DVE starts early
            sync.dma_start(out=sb_blob[:, 0:R], in_=blob[:, 0:R]).then_inc(
                dma_sem, 16
            )
            sync.dma_start(
                out=sb_blob[:, R : 2 * R + NPAR], in_=blob[:, R : 2 * R + NPAR]
            ).then_inc(dma_sem, 16)
            sync.wait_ge(dma_sem, 48)  # in x2 + act-issued out

        @block.scalar
        def _(act):
            act.memzero(zt[:])
            act.drain()
            # col-11 per-slot sums via Relu accumulation, hidden under the
            # second DMA: s0 = sum(relu(vs0)), s1 = sum(relu(-vs0))
            act.wait_ge(dma_sem, 16)
            act.activation(
                out=junkA[:], in_=vs0, func=ACT.Relu, bias=zt[:], accum_out=sab[:, 0:1]
            )
            act.activation(
                out=junkA[:],
                in_=vs0,
                func=ACT.Relu,
                bias=zt[:],
                scale=-1.0,
                accum_out=sab[:, 1:2],
            ).then_inc(act_sem, 1)
            # final sigmoid after the DVE tail, then write scores out
            # (drain so the DMA engine cannot read sc before sigmoid lands)
            act.wait_ge(dve_sem, 1)
            act.activation(out=sc[:], in_=z[:], func=ACT.Sigmoid, bias=zt[:])
            act.drain()
            act.dma_start(out=out[:], in_=sc[:]).then_inc(dma_sem, 16)

        @block.vector
        def _(dve):
            # raw-bass engine ops do NOT see the immediately preceding op's
            # SBUF writes; drain between dependent ops
            D = dve.drain
            dve.memset(ones2[:], 1.0)
            dve.memset(t1[:, 0, H + 1 : H + 2], 1.0)
            dve.memset(t1[:, 1, H + 1 : H + 2], 1.0)
            dve.wait_ge(dma_sem, 16)

            # reductions as accumulating tensor_scalar (2x_2p DVE perf mode;
            # accum variant: out = in op0 s1, accum_out = reduce(out, op1));
            # counts from the sign of vs0
            dve.tensor_scalar(
                junk0[:], vs0, 0.0, None, OP.is_gt, OP.add, accum_out=cnt2[:, 0:1]
            )
            dve.tensor_scalar(
                junk0[:], vs0, 0.0, None, OP.is_lt, OP.add, accum_out=cnt2[:, 1:2]
            )
            D()
            # den = max(c,1)*-1 -> rec = -1/max(c,1); pen for empty graphs
            # (these run while the second DMA is still in flight)
            dve.tensor_scalar(den[:], cnt2[:], 1.0, -1.0, OP.max, OP.mult)
            dve.tensor_scalar(pen[:], cnt2[:], 0.0, 1e30, OP.is_le, OP.mult)
            D()
            dve.reciprocal(rec[:], den[:])
            # col-24 sums once the second DMA lands: s0 = sum(max(vs1,0)),
            # s1 = -sum(min(vs1,0)) -- negation handled via the subtract-form
            # af and the host-negated W1[:,1] copy
            dve.wait_ge(dma_sem, 32)
            dve.tensor_scalar(
                junk0[:], vs1, 0.0, None, OP.max, OP.add, accum_out=sab[:, 2:3]
            )
            dve.tensor_scalar(
                junk0[:], vs1, 0.0, None, OP.min, OP.add, accum_out=sab[:, 3:4]
            )
            dve.wait_ge(act_sem, 1)
            D()
            # access features as per-partition scalars, one level:
            #   af[:,0] = af_c0w0 = s0c0*rec0 + 1
            #   af[:,1] = af_c0w1 = s1c0*rec1 + 1
            #   af[:,2] = af_c1w0 = s0c1*rec0 + 1
            #   af[:,3] = -af_c1w1 = (-s1c1)*rec1 - 1   (sab3 holds -s1c1;
            #            compensated by the host-negated W1[:,1] copy below)
            one1 = ones2[:, 0:1]
            dve.scalar_tensor_tensor(
                af[:, 0:3:2], sab[:, 0:3:2], rec[:, 0:1], ones2[:], OP.mult, OP.add
            )
            dve.scalar_tensor_tensor(
                af[:, 1:2], sab[:, 1:2], rec[:, 1:2], one1, OP.mult, OP.add
            )
            dve.scalar_tensor_tensor(
                af[:, 3:4], sab[:, 3:4], rec[:, 1:2], one1, OP.mult, OP.subtract
            )
            D()
            # hidden layer per slot w, hidden dim along the free axis:
            #   v_w = af_c1w*W1[:,1] + b1 ; u_w = af_c0w*W1[:,0] + v_w
            w1c0r = sb_blob[:, q : q + H]
            w1c1r = sb_blob[:, q + H : q + 2 * H]
            w1c1n = sb_blob[:, q + 2 * H : q + 3 * H]  # -W1[:,1]
            b1r = sb_blob[:, q + 3 * H : q + 4 * H]
            w2r = sb_blob[:, q + 4 * H : q + 5 * H]
            dve.scalar_tensor_tensor(
                t0[:, 0, :], w1c1r, af[:, 2:3], b1r, OP.mult, OP.add
            )
            dve.scalar_tensor_tensor(
                t0[:, 1, :], w1c1n, af[:, 3:4], b1r, OP.mult, OP.add
            )
            D()
            dve.scalar_tensor_tensor(
                t1[:, 0, 0:H], w1c0r, af[:, 0:1], t0[:, 0, :], OP.mult, OP.add
            )
            dve.scalar_tensor_tensor(
                t1[:, 1, 0:H], w1c0r, af[:, 1:2], t0[:, 1, :], OP.mult, OP.add
            )
            # two extra hidden units ride along: u[32] = +1e30*empty (W2
            # coefficient -1 -> the empty-graph penalty) and u[33] = 1
            # (W2 coefficient b2 -> the bias); both survive the relu
            dve.tensor_copy(t1[:, 0, H : H + 1], pen[:, 0:1])
            dve.tensor_copy(t1[:, 1, H : H + 1], pen[:, 1:2])
            D()
            # z = sum_j relu(u_j) * w2ext_j  (w2ext = [W2, -1, b2])
            w2e = sb_blob[:, q + 4 * H : q + 5 * H + 2]
            dve.scalar_tensor_tensor(
                hws[:, 0, :], t1[:, 0, :], 0.0, w2e, OP.max, OP.mult,
                accum_out=z[:, 0:1],
            )
            dve.scalar_tensor_tensor(
                hws[:, 1, :], t1[:, 1, :], 0.0, w2e, OP.max, OP.mult,
                accum_out=z[:, 1:2],
            ).then_inc(dve_sem, 1)

    return nc


def kernel(
    node_features,
    batch,
    graph_embedding=None,
    W1=None,
    b1=None,
    W2=None,
    b2=None,
    num_graphs=None,
    **_unused,
):
    global LAST_EXEC_TIME_NS, LAST_PROFILE

    node_features = np.asarray(node_features)
    batch = np.asarray(batch)
    N = batch.shape[0]

    # CSR-style boundaries of the sorted index (pure index prep, O(G log N))
    bounds = np.searchsorted(batch, np.arange(G + 1, dtype=batch.dtype))
    counts = np.diff(bounds)
    # pair the largest-count graph with the smallest etc. so the padded
    # row length R tracks 2*mean instead of the worst adjacent pair
    order = np.argsort(counts, kind="stable")
    ga = order[: G // 2]  # slot-0 graph of pair j
    gb = order[G // 2 :][::-1]  # slot-1 graph of pair j
    R = int(np.ceil((counts[ga] + counts[gb]).max() / 8.0)) * 8

    # signed values: slot-1 nodes negated; exact zeros nudged so the
    # device can recover counts from the sign (values are >= 0 here)
    slot_of_graph = np.empty(G, np.int8)
    slot_of_graph[ga] = 0
    slot_of_graph[gb] = 1
    sgn = 1.0 - 2.0 * slot_of_graph[batch].astype(np.float32)
    colsT = np.empty((2, N), np.float32)

    # strided 512MB read; numpy slice copies release the GIL, so chunked
    # threads overlap the memory stalls
    def _extract(lo, hi):
        colsT[0, lo:hi] = node_features[lo:hi, MODIFIER_COL]
        colsT[1, lo:hi] = node_features[lo:hi, OWNER_COL]

    nchunk = 8
    bounds_t = [N * i // nchunk for i in range(nchunk + 1)]
    with ThreadPoolExecutor(nchunk) as ex:
        list(ex.map(lambda i: _extract(bounds_t[i], bounds_t[i + 1]), range(nchunk)))
    assert colsT[0].min() >= 0.0, "sign encoding needs non-negative col values"
    zz = colsT[0] == 0.0
    if zz.any():
        colsT[0, zz] = 1e-30
    vsT = colsT * sgn

    W1 = np.asarray(W1, np.float32)  # [32, 2]
    W2 = np.asarray(W2, np.float32)  # [1, 32]
    b1 = np.asarray(b1, np.float32)
    b2v = np.float32(np.asarray(b2, np.float32).reshape(()))

    blob_all = np.zeros((N_CORES, P, 2 * R + NPAR), np.float32)
    q = 2 * R
    blob_all[:, :, q + 0 * H : q + 1 * H] = W1[:, 0]
    blob_all[:, :, q + 1 * H : q + 2 * H] = W1[:, 1]
    blob_all[:, :, q + 2 * H : q + 3 * H] = -W1[:, 1]
    blob_all[:, :, q + 3 * H : q + 4 * H] = b1
    blob_all[:, :, q + 4 * H : q + 5 * H] = W2[0]
    blob_all[:, :, q + 5 * H] = -1.0
    blob_all[:, :, q + 5 * H + 1] = b2v
    flat = blob_all.reshape(N_CORES * P, 2 * R + NPAR)
    for j in range(N_CORES * P):
        o = 0
        for g in (ga[j], gb[j]):
            s, t = bounds[g], bounds[g + 1]
            if t > s:
                L = t - s
                flat[j, o : o + L] = vsT[0, s:t]
                flat[j, R + o : R + o + L] = vsT[1, s:t]
                o += L

    in_maps = [{"blob": blob_all[c]} for c in range(N_CORES)]

    global LAST_R
    LAST_R = R
    nc = _build_bass(R)
    trace = os.environ.get("BASS_KERNEL_PROFILE") == "1"
    res = run_bass_kernel_spmd(nc, in_maps, list(range(N_CORES)), trace=trace)
    LAST_EXEC_TIME_NS = getattr(res, "exec_time_ns", None)
    LAST_PROFILE = getattr(res, "profile_json", None)

    # pair j = (core j//P, partition j%P); slot 0 -> graph ga[j], slot 1 -> gb[j]
    allres = np.concatenate(
        [res.results[c]["score_out"] for c in range(N_CORES)], axis=0
    )  # [1024, 2]
    scores = np.empty((G,), np.float32)
    scores[ga] = allres[:, 0]
    scores[gb] = allres[:, 1]
    return scores


# revision 85
# speedup vs baseline: 1.0559x; 1.0174x over previous
"""Trainium2 Bass kernel for nn_AccessControlHead (segment_reduce).

Reference computation: per-graph means of node_features columns 11 and 24
(segment-sum over a sorted batch index, G=2048 graphs), then a tiny MLP
  score = sigmoid(relu((1-means) @ W1.T + b1) @ W2.T + b2), 0 for empty graphs.

Strategy: batch is sorted, so each graph's nodes are contiguous. We shard
graph-aligned: 8 cores x 128 partitions x 2 graphs per partition = 2048
graphs. Graphs are paired largest-with-smallest by node count so the
padded row length R tracks 2x the mean count instead of the worst pair.
The host packs, for every partition, the node values of its two graphs
(padded to a common length R) with the graph slot encoded in the SIGN:
slot 0 nodes keep v, slot 1 nodes carry -v (v > 0 is guaranteed by a
host-side nudge of exact zeros). Each core recovers, per partition:
  counts  c0 = #(vs0 > 0), c1 = #(vs0 < 0)   (DVE compares, fused accum)
  sums    s0 = sum(relu(vs)), s1 = sum(relu(-vs))
with the six reductions expressed as accumulating tensor_scalar /
activation ops: col-11 sums run on ScalarE (hidden under the second half
of the input DMA) while DVE does the counts; DVE does the col-24 sums.
The tiny MLP runs per slot with scalar_tensor_tensor chains (hidden dim
on the free axis) and one ScalarE sigmoid. No collectives: every core
fully owns 256 consecutive graphs. Empty-graph masking adds -1e30 to the
sigmoid argument (sigmoid(-1e30) == 0.0).

Raw-bass notes (hard-won): engine ops do NOT observe the immediately
preceding op's SBUF writes -> explicit drain() between dependent ops;
PE/DVE reject partition offsets not in {0,32,64}; 3-D tensor_reduce
mis-reduces on HW; plain tensor_reduce runs ~4x slower than the 2x_2p
accumulating tensor_scalar form.
"""

import os
from concurrent.futures import ThreadPoolExecutor
from contextlib import ExitStack

import numpy as np

import concourse.bass as bass
from concourse import mybir
from concourse.bass_utils import run_bass_kernel_spmd

G = 2048
N_CORES = 8
P = 128  # partitions per core
H = 32  # MLP hidden dim
MODIFIER_COL = 11
OWNER_COL = 24
NPAR = 163  # params tail: W1c0 | W1c1 | -W1c1 | b1 | [W2, -1, b2]

F32 = mybir.dt.float32

# populated when BASS_KERNEL_PROFILE=1 so a harness can report HW time
LAST_EXEC_TIME_NS = None
LAST_PROFILE = None
LAST_R = 1088


def _bcast(ap: bass.AP, axis: int, n: int) -> bass.AP:
    """Insert a stride-0 broadcast dim of size n at position `axis` of an AP."""
    pat = list(ap.ap)
    pat.insert(axis, [0, n])
    return bass.AP(tensor=ap.tensor, offset=ap.offset, ap=pat)


def _lean_bass() -> bass.Bass:
    """Bass() without the const-AP memsets + all-engine barrier preamble
    (~0.7us). The kernel supplies its own zero bias tile for activations,
    so the const APs are never read."""
    orig_memset = bass.BassSharedVectorInterface.memset
    orig_barrier = bass.Bass.all_engine_barrier
    bass.BassSharedVectorInterface.memset = lambda self, ap, c: None
    bass.Bass.all_engine_barrier = lambda self, *a, **k: None
    try:
        return bass.Bass()
    finally:
        bass.BassSharedVectorInterface.memset = orig_memset
        bass.Bass.all_engine_barrier = orig_barrier


def _build_bass(R: int) -> bass.Bass:
    nc = _lean_bass()

    # input blob per partition row:
    #   [:, 0:R]        signed node values col 11 (vs0), zero-padded
    #   [:, R:2R]       signed node values col 24 (vs1)
    #   [:, 2R+0:+32]    W1[:,0] (replicated on every partition)
    #   [:, 2R+32:+64]   W1[:,1]
    #   [:, 2R+64:+96]   -W1[:,1] (for the w=1 chain: its af_c1 is negated)
    #   [:, 2R+96:+128]  b1
    #   [:, 2R+128:+160] W2[0,:]
    #   [:, 2R+160:+162] [-1, b2] (extra W2 coefficients for the penalty
    #                    and bias hidden units)
    blob = nc.dram_tensor("blob", [P, 2 * R + NPAR], F32, kind="ExternalInput")
    out = nc.dram_tensor("score_out", [P, 2], F32, kind="ExternalOutput")

    AX = mybir.AxisListType.X
    OP = mybir.AluOpType
    ACT = mybir.ActivationFunctionType
    q = 2 * R

    with ExitStack() as ctx:

        def sb(name, shape):
            return ctx.enter_context(nc.sbuf_tensor(name, shape, F32))

        block = ctx.enter_context(nc.Block())
        dma_sem = ctx.enter_context(nc.semaphore("dma_sem"))
        dve_sem = ctx.enter_context(nc.semaphore("dve_sem"))
        act_sem = ctx.enter_context(nc.semaphore("act_sem"))

        sb_blob = sb("sb_blob", [P, 2 * R + NPAR])
        junk0 = sb("junk0", [P, R])
        junkA = sb("junkA", [P, R])
        cnt2 = sb("cnt2", [P, 2])  # [c0, c1]
        sab = sb("sab", [P, 4])  # [s0_c0, s1_c0, s0_c1, s1_c1]
        ones2 = sb("ones2", [P, 2])
        den = sb("den", [P, 2])
        rec = sb("rec", [P, 2])
        af = sb("af", [P, 4])  # [af_c0w0, af_c0w1, af_c1w0, af_c1w1]
        t0 = sb("t0", [P, 2, H])
        t1 = sb("t1", [P, 2, H + 2])
        hws = sb("hws", [P, 2, H + 2])
        zs = sb("zs", [P, 2])
        pen = sb("pen", [P, 2])
        z = sb("z", [P, 2])
        sc = sb("sc", [P, 2])
        zt = sb("zt", [P, 1])  # ACT-owned zero bias (replaces const APs)

        vs0 = sb_blob[:, 0:R]
        vs1 = sb_blob[:, R : 2 * R]

        @block.sync
        def _(sync):
            # split the input DMA: col-11 region first so DVE starts early
            sync.dma_start(out=sb_blob[:, 0:R], in_=blob[:, 0:R]).then_inc(
                dma_sem, 16
            )
            sync.dma_start(
                out=sb_blob[:, R : 2 * R + NPAR], in_=blob[:, R : 2 * R + NPAR]
            ).then_inc(dma_sem, 16)
            sync.wait_ge(dma_sem, 48)  # in x2 + act-issued out

        @block.scalar
        def _(act):
            act.memzero(zt[:])
            act.drain()
            # col-11 per-slot sums via Relu accumulation, hidden under the
            # second DMA: s0 = sum(relu(vs0)), s1 = sum(relu(-vs0))
            act.wait_ge(dma_sem, 16)
            act.activation(
                out=junkA[:], in_=vs0, func=ACT.Relu, bias=zt[:], accum_out=sab[:, 0:1]
            )
            act.activation(
                out=junkA[:],
                in_=vs0,
                func=ACT.Relu,
                bias=zt[:],
                scale=-1.0,
                accum_out=sab[:, 1:2],
            ).then_inc(act_sem, 1)
            # final sigmoid after the DVE tail, then write scores out.
            # No drain needed: the HWDGE first SBUF read trails the issue by
            # >=~600ns while the [P,2] sigmoid's write lands within ~80ns.
            act.wait_ge(dve_sem, 1)
            act.activation(out=sc[:], in_=z[:], func=ACT.Sigmoid, bias=zt[:])
            act.dma_start(out=out[:], in_=sc[:]).then_inc(dma_sem, 16)

        @block.vector
        def _(dve):
            # raw-bass engine ops do NOT see the immediately preceding op's
            # SBUF writes; drain between dependent ops
            D = dve.drain
            dve.memset(ones2[:], 1.0)
            dve.memset(t1[:, 0, H + 1 : H + 2], 1.0)
            dve.memset(t1[:, 1, H + 1 : H + 2], 1.0)
            dve.wait_ge(dma_sem, 16)

            # reductions as accumulating tensor_scalar (2x_2p DVE perf mode;
            # accum variant: out = in op0 s1, accum_out = reduce(out, op1));
            # counts from the sign of vs0
            dve.tensor_scalar(
                junk0[:], vs0, 0.0, None, OP.is_gt, OP.add, accum_out=cnt2[:, 0:1]
            )
            dve.tensor_scalar(
                junk0[:], vs0, 0.0, None, OP.is_lt, OP.add, accum_out=cnt2[:, 1:2]
            )
            D()
            # den = max(c,1)*-1 -> rec = -1/max(c,1); pen for empty graphs
            # (these run while the second DMA is still in flight)
            dve.tensor_scalar(den[:], cnt2[:], 1.0, -1.0, OP.max, OP.mult)
            dve.tensor_scalar(pen[:], cnt2[:], 0.0, 1e30, OP.is_le, OP.mult)
            D()
            dve.reciprocal(rec[:], den[:])
            # col-24 sums once the second DMA lands: s0 = sum(max(vs1,0)),
            # s1 = -sum(min(vs1,0)) -- negation handled via the subtract-form
            # af and the host-negated W1[:,1] copy
            dve.wait_ge(dma_sem, 32)
            dve.tensor_scalar(
                junk0[:], vs1, 0.0, None, OP.max, OP.add, accum_out=sab[:, 2:3]
            )
            dve.tensor_scalar(
                junk0[:], vs1, 0.0, None, OP.min, OP.add, accum_out=sab[:, 3:4]
            )
            dve.wait_ge(act_sem, 1)
            D()
            # access features as per-partition scalars, one level:
            #   af[:,0] = af_c0w0 = s0c0*rec0 + 1
            #   af[:,1] = af_c0w1 = s1c0*rec1 + 1
            #   af[:,2] = af_c1w0 = s0c1*rec0 + 1
            #   af[:,3] = -af_c1w1 = (-s1c1)*rec1 - 1   (sab3 holds -s1c1;
            #            compensated by the host-negated W1[:,1] copy below)
            one1 = ones2[:, 0:1]
            dve.scalar_tensor_tensor(
                af[:, 0:3:2], sab[:, 0:3:2], rec[:, 0:1], ones2[:], OP.mult, OP.add
            )
            dve.scalar_tensor_tensor(
                af[:, 1:2], sab[:, 1:2], rec[:, 1:2], one1, OP.mult, OP.add
            )
            dve.scalar_tensor_tensor(
                af[:, 3:4], sab[:, 3:4], rec[:, 1:2], one1, OP.mult, OP.subtract
            )
            D()
            # hidden layer per slot w, hidden dim along the free axis:
            #   v_w = af_c1w*W1[:,1] + b1 ; u_w = af_c0w*W1[:,0] + v_w
            w1c0r = sb_blob[:, q : q + H]
            w1c1r = sb_blob[:, q + H : q + 2 * H]
            w1c1n = sb_blob[:, q + 2 * H : q + 3 * H]  # -W1[:,1]
            b1r = sb_blob[:, q + 3 * H : q + 4 * H]
            w2r = sb_blob[:, q + 4 * H : q + 5 * H]
            dve.scalar_tensor_tensor(
                t0[:, 0, :], w1c1r, af[:, 2:3], b1r, OP.mult, OP.add
            )
            dve.scalar_tensor_tensor(
                t0[:, 1, :], w1c1n, af[:, 3:4], b1r, OP.mult, OP.add
            )
            D()
            dve.scalar_tensor_tensor(
                t1[:, 0, 0:H], w1c0r, af[:, 0:1], t0[:, 0, :], OP.mult, OP.add
            )
            dve.scalar_tensor_tensor(
                t1[:, 1, 0:H], w1c0r, af[:, 1:2], t0[:, 1, :], OP.mult, OP.add
            )
            # two extra hidden units ride along: u[32] = +1e30*empty (W2
            # coefficient -1 -> the empty-graph penalty) and u[33] = 1
            # (W2 coefficient b2 -> the bias); both survive the relu
            dve.tensor_copy(t1[:, 0, H : H + 1], pen[:, 0:1])
            dve.tensor_copy(t1[:, 1, H : H + 1], pen[:, 1:2])
            D()
            # z = sum_j relu(u_j) * w2ext_j  (w2ext = [W2, -1, b2])
            w2e = sb_blob[:, q + 4 * H : q + 5 * H + 2]
            dve.scalar_tensor_tensor(
                hws[:, 0, :], t1[:, 0, :], 0.0, w2e, OP.max, OP.mult,
                accum_out=z[:, 0:1],
            )
            dve.scalar_tensor_tensor(
                hws[:, 1, :], t1[:, 1, :], 0.0, w2e, OP.max, OP.mult,
                accum_out=z[:, 1:2],
            ).then_inc(dve_sem, 1)

    return nc


def kernel(
    node_features,
    batch,
    graph_embedding=None,
    W1=None,
    b1=None,
    W2=None,
    b2=None,
    num_graphs=None,
    **_unused,
):
    global LAST_EXEC_TIME_NS, LAST_PROFILE

    node_features = np.asarray(node_features)
    batch = np.asarray(batch)
    N = batch.shape[0]

    # CSR-style boundaries of the sorted index (pure index prep, O(G log N))
    bounds = np.searchsorted(batch, np.arange(G + 1, dtype=batch.dtype))
    counts = np.diff(bounds)
    # pair the largest-count graph with the smallest etc. so the padded
    # row length R tracks 2*mean instead of the worst adjacent pair
    order = np.argsort(counts, kind="stable")
    ga = order[: G // 2]  # slot-0 graph of pair j
    gb = order[G // 2 :][::-1]  # slot-1 graph of pair j
    R = int(np.ceil((counts[ga] + counts[gb]).max() / 8.0)) * 8

    # signed values: slot-1 nodes negated; exact zeros nudged so the
    # device can recover counts from the sign (values are >= 0 here)
    slot_of_graph = np.empty(G, np.int8)
    slot_of_graph[ga] = 0
    slot_of_graph[gb] = 1
    sgn = 1.0 - 2.0 * slot_of_graph[batch].astype(np.float32)
    colsT = np.empty((2, N), np.float32)

    # strided 512MB read; numpy slice copies release the GIL, so chunked
    # threads overlap the memory stalls
    def _extract(lo, hi):
        colsT[0, lo:hi] = node_features[lo:hi, MODIFIER_COL]
        colsT[1, lo:hi] = node_features[lo:hi, OWNER_COL]

    nchunk = 8
    bounds_t = [N * i // nchunk for i in range(nchunk + 1)]
    with ThreadPoolExecutor(nchunk) as ex:
        list(ex.map(lambda i: _extract(bounds_t[i], bounds_t[i + 1]), range(nchunk)))
    assert colsT[0].min() >= 0.0, "sign encoding needs non-negative col values"
    zz = colsT[0] == 0.0
    if zz.any():
        colsT[0, zz] = 1e-30
    vsT = colsT * sgn

    W1 = np.asarray(W1, np.float32)  # [32, 2]
    W2 = np.asarray(W2, np.float32)  # [1, 32]
    b1 = np.asarray(b1, np.float32)
    b2v = np.float32(np.asarray(b2, np.float32).reshape(()))

    blob_all = np.zeros((N_CORES, P, 2 * R + NPAR), np.float32)
    q = 2 * R
    blob_all[:, :, q + 0 * H : q + 1 * H] = W1[:, 0]
    blob_all[:, :, q + 1 * H : q + 2 * H] = W1[:, 1]
    blob_all[:, :, q + 2 * H : q + 3 * H] = -W1[:, 1]
    blob_all[:, :, q + 3 * H : q + 4 * H] = b1
    blob_all[:, :, q + 4 * H : q + 5 * H] = W2[0]
    blob_all[:, :, q + 5 * H] = -1.0
    blob_all[:, :, q + 5 * H + 1] = b2v
    flat = blob_all.reshape(N_CORES * P, 2 * R + NPAR)
    for j in range(N_CORES * P):
        o = 0
        for g in (ga[j], gb[j]):
            s, t = bounds[g], bounds[g + 1]
            if t > s:
                L = t - s
                flat[j, o : o + L] = vsT[0, s:t]
                flat[j, R + o : R + o + L] = vsT[1, s:t]
                o += L

    in_maps = [{"blob": blob_all[c]} for c in range(N_CORES)]

    global LAST_R
    LAST_R = R
    nc = _build_bass(R)
    trace = os.environ.get("BASS_KERNEL_PROFILE") == "1"
    res = run_bass_kernel_spmd(nc, in_maps, list(range(N_CORES)), trace=trace)
    LAST_EXEC_TIME_NS = getattr(res, "exec_time_ns", None)
    LAST_PROFILE = getattr(res, "profile_json", None)

    # pair j = (core j//P, partition j%P); slot 0 -> graph ga[j], slot 1 -> gb[j]
    allres = np.concatenate(
        [res.results[c]["score_out"] for c in range(N_CORES)], axis=0
    )  # [1024, 2]
    scores = np.empty((G,), np.float32)
    scores[ga] = allres[:, 0]
    scores[gb] = allres[:, 1]
    return scores


# revision 86
# speedup vs baseline: 1.0751x; 1.0182x over previous
"""Trainium2 Bass kernel for nn_AccessControlHead (segment_reduce).

Reference computation: per-graph means of node_features columns 11 and 24
(segment-sum over a sorted batch index, G=2048 graphs), then a tiny MLP
  score = sigmoid(relu((1-means) @ W1.T + b1) @ W2.T + b2), 0 for empty graphs.

Strategy: batch is sorted, so each graph's nodes are contiguous. We shard
graph-aligned: 8 cores x 128 partitions x 2 graphs per partition = 2048
graphs. Graphs are paired largest-with-smallest by node count so the
padded row length R tracks 2x the mean count instead of the worst pair.
The host packs, for every partition, the node values of its two graphs
(padded to a common length R) with the graph slot encoded in the SIGN:
slot 0 nodes keep v, slot 1 nodes carry -v (v > 0 is guaranteed by a
host-side nudge of exact zeros). Each core recovers, per partition:
  counts  c0 = #(vs0 > 0), c1 = #(vs0 < 0)   (DVE compares, fused accum)
  sums    s0 = sum(relu(vs)), s1 = sum(relu(-vs))
with the six reductions expressed as accumulating tensor_scalar /
activation ops: col-11 sums run on ScalarE (hidden under the second half
of the input DMA) while DVE does the counts; DVE does the col-24 sums.
The tiny MLP runs per slot with scalar_tensor_tensor chains (hidden dim
on the free axis) and one ScalarE sigmoid. No collectives: every core
fully owns 256 consecutive graphs. Empty-graph masking adds -1e30 to the
sigmoid argument (sigmoid(-1e30) == 0.0).

Raw-bass notes (hard-won): engine ops do NOT observe the immediately
preceding op's SBUF writes -> explicit drain() between dependent ops;
PE/DVE reject partition offsets not in {0,32,64}; 3-D tensor_reduce
mis-reduces on HW; plain tensor_reduce runs ~4x slower than the 2x_2p
accumulating tensor_scalar form.
"""

import os
from concurrent.futures import ThreadPoolExecutor
from contextlib import ExitStack

import numpy as np

import concourse.bass as bass
from concourse import mybir
from concourse.bass_utils import run_bass_kernel_spmd

G = 2048
N_CORES = 8
P = 128  # partitions per core
H = 32  # MLP hidden dim
MODIFIER_COL = 11
OWNER_COL = 24
NPAR = 163  # params tail: W1c0 | W1c1 | -W1c1 | b1 | [W2, -1, b2]

F32 = mybir.dt.float32

# populated when BASS_KERNEL_PROFILE=1 so a harness can report HW time
LAST_EXEC_TIME_NS = None
LAST_PROFILE = None
LAST_R = 1088


def _bcast(ap: bass.AP, axis: int, n: int) -> bass.AP:
    """Insert a stride-0 broadcast dim of size n at position `axis` of an AP."""
    pat = list(ap.ap)
    pat.insert(axis, [0, n])
    return bass.AP(tensor=ap.tensor, offset=ap.offset, ap=pat)


def _lean_bass() -> bass.Bass:
    """Bass() without the const-AP memsets + all-engine barrier preamble
    (~0.7us). The kernel supplies its own zero bias tile for activations,
    so the const APs are never read."""
    orig_memset = bass.BassSharedVectorInterface.memset
    orig_barrier = bass.Bass.all_engine_barrier
    bass.BassSharedVectorInterface.memset = lambda self, ap, c: None
    bass.Bass.all_engine_barrier = lambda self, *a, **k: None
    try:
        return bass.Bass()
    finally:
        bass.BassSharedVectorInterface.memset = orig_memset
        bass.Bass.all_engine_barrier = orig_barrier


def _build_bass(R: int) -> bass.Bass:
    nc = _lean_bass()

    # input blob per partition row:
    #   [:, 0:R]        signed node values col 11 (vs0), zero-padded
    #   [:, R:2R]       signed node values col 24 (vs1)
    #   [:, 2R+0:+32]    W1[:,0] (replicated on every partition)
    #   [:, 2R+32:+64]   W1[:,1]
    #   [:, 2R+64:+96]   -W1[:,1] (for the w=1 chain: its af_c1 is negated)
    #   [:, 2R+96:+128]  b1
    #   [:, 2R+128:+160] W2[0,:]
    #   [:, 2R+160:+162] [-1, b2] (extra W2 coefficients for the penalty
    #                    and bias hidden units)
    blob = nc.dram_tensor("blob", [P, 2 * R + NPAR], F32, kind="ExternalInput")
    out = nc.dram_tensor("score_out", [P, 2], F32, kind="ExternalOutput")

    AX = mybir.AxisListType.X
    OP = mybir.AluOpType
    ACT = mybir.ActivationFunctionType
    q = 2 * R

    with ExitStack() as ctx:

        def sb(name, shape):
            return ctx.enter_context(nc.sbuf_tensor(name, shape, F32))

        block = ctx.enter_context(nc.Block())
        dma_sem = ctx.enter_context(nc.semaphore("dma_sem"))
        dve_sem = ctx.enter_context(nc.semaphore("dve_sem"))
        act_sem = ctx.enter_context(nc.semaphore("act_sem"))

        sb_blob = sb("sb_blob", [P, 2 * R + NPAR])
        junk0 = sb("junk0", [P, R])
        junkA = sb("junkA", [P, R])
        cnt2 = sb("cnt2", [P, 2])  # [c0, c1]
        sab = sb("sab", [P, 4])  # [s0_c0, s1_c0, s0_c1, s1_c1]
        ones2 = sb("ones2", [P, 2])
        den = sb("den", [P, 2])
        rec = sb("rec", [P, 2])
        af = sb("af", [P, 4])  # [af_c0w0, af_c0w1, af_c1w0, af_c1w1]
        t0 = sb("t0", [P, 2, H])
        t1 = sb("t1", [P, 2, H + 2])
        hws = sb("hws", [P, 2, H + 2])
        zs = sb("zs", [P, 2])
        pen = sb("pen", [P, 2])
        z = sb("z", [P, 2])
        sc = sb("sc", [P, 2])
        zt = sb("zt", [P, 1])  # ACT-owned zero bias (replaces const APs)

        vs0 = sb_blob[:, 0:R]
        vs1 = sb_blob[:, R : 2 * R]

        @block.sync
        def _(sync):
            # split the input DMA: col-11 region first so DVE starts early
            sync.dma_start(out=sb_blob[:, 0:R], in_=blob[:, 0:R]).then_inc(
                dma_sem, 16
            )
            sync.dma_start(
                out=sb_blob[:, R : 2 * R + NPAR], in_=blob[:, R : 2 * R + NPAR]
            ).then_inc(dma_sem, 16)
            sync.wait_ge(dma_sem, 48)  # in x2 + act-issued out

        @block.scalar
        def _(act):
            act.memzero(zt[:])
            act.drain()
            # col-11 per-slot sums via Relu accumulation, hidden under the
            # second DMA: s0 = sum(relu(vs0)), s1 = sum(relu(-vs0))
            act.wait_ge(dma_sem, 16)
            act.activation(
                out=junkA[:], in_=vs0, func=ACT.Relu, bias=zt[:], accum_out=sab[:, 0:1]
            )
            act.activation(
                out=junkA[:],
                in_=vs0,
                func=ACT.Relu,
                bias=zt[:],
                scale=-1.0,
                accum_out=sab[:, 1:2],
            ).then_inc(act_sem, 1)
            # final sigmoid after the DVE tail, then write scores out.
            # No drain needed: the HWDGE first SBUF read trails the issue by
            # >=~600ns while the [P,2] sigmoid's write lands within ~80ns.
            act.wait_ge(dve_sem, 1)
            act.activation(out=sc[:], in_=z[:], func=ACT.Sigmoid, bias=zt[:])
            act.dma_start(out=out[:], in_=sc[:]).then_inc(dma_sem, 16)

        @block.vector
        def _(dve):
            # raw-bass engine ops do NOT see the immediately preceding op's
            # SBUF writes; drain between dependent ops
            D = dve.drain
            dve.memset(ones2[:], 1.0)
            dve.memset(t1[:, 0, H + 1 : H + 2], 1.0)
            dve.memset(t1[:, 1, H + 1 : H + 2], 1.0)
            dve.wait_ge(dma_sem, 16)

            # reductions as accumulating tensor_scalar (2x_2p DVE perf mode;
            # accum variant: out = in op0 s1, accum_out = reduce(out, op1));
            # counts from the sign of vs0
            dve.tensor_scalar(
                junk0[:], vs0, 0.0, None, OP.is_gt, OP.add, accum_out=cnt2[:, 0:1]
            )
            dve.tensor_scalar(
                junk0[:], vs0, 0.0, None, OP.is_lt, OP.add, accum_out=cnt2[:, 1:2]
            )
            D()
            # den = max(c,1)*-1 -> rec = -1/max(c,1); pen for empty graphs
            # (these run while the second DMA is still in flight)
            dve.tensor_scalar(den[:], cnt2[:], 1.0, -1.0, OP.max, OP.mult)
            dve.tensor_scalar(pen[:], cnt2[:], 0.0, 1e30, OP.is_le, OP.mult)
            D()
            dve.reciprocal(rec[:], den[:])
            # col-24 sums once the second DMA lands: s0 = sum(max(vs1,0)),
            # s1 = -sum(min(vs1,0)) -- negation handled via the subtract-form
            # af and the host-negated W1[:,1] copy
            dve.wait_ge(dma_sem, 32)
            dve.tensor_scalar(
                junk0[:], vs1, 0.0, None, OP.max, OP.add, accum_out=sab[:, 2:3]
            )
            dve.tensor_scalar(
                junk0[:], vs1, 0.0, None, OP.min, OP.add, accum_out=sab[:, 3:4]
            )
            dve.wait_ge(act_sem, 1)
            # no drain: the min pass is a big op (>266ns) whose implicit
            # pipe-drain commits its writes before the next issue; the af
            # ops are additionally ordered so each read sits >=2 ops back
            #   af[:,0] = af_c0w0 = s0c0*rec0 + 1
            #   af[:,1] = af_c0w1 = s1c0*rec1 + 1
            #   af[:,2] = af_c1w0 = s0c1*rec0 + 1
            #   af[:,3] = -af_c1w1 = (-s1c1)*rec1 - 1   (sab3 holds -s1c1;
            #            compensated by the host-negated W1[:,1] copy below)
            one1 = ones2[:, 0:1]
            dve.scalar_tensor_tensor(
                af[:, 0:3:2], sab[:, 0:3:2], rec[:, 0:1], ones2[:], OP.mult, OP.add
            )
            dve.scalar_tensor_tensor(
                af[:, 1:2], sab[:, 1:2], rec[:, 1:2], one1, OP.mult, OP.add
            )
            dve.scalar_tensor_tensor(
                af[:, 3:4], sab[:, 3:4], rec[:, 1:2], one1, OP.mult, OP.subtract
            )
            D()
            # hidden layer per slot w, hidden dim along the free axis:
            #   v_w = af_c1w*W1[:,1] + b1 ; u_w = af_c0w*W1[:,0] + v_w
            w1c0r = sb_blob[:, q : q + H]
            w1c1r = sb_blob[:, q + H : q + 2 * H]
            w1c1n = sb_blob[:, q + 2 * H : q + 3 * H]  # -W1[:,1]
            b1r = sb_blob[:, q + 3 * H : q + 4 * H]
            w2r = sb_blob[:, q + 4 * H : q + 5 * H]
            dve.scalar_tensor_tensor(
                t0[:, 0, :], w1c1r, af[:, 2:3], b1r, OP.mult, OP.add
            )
            dve.scalar_tensor_tensor(
                t0[:, 1, :], w1c1n, af[:, 3:4], b1r, OP.mult, OP.add
            )
            D()
            dve.scalar_tensor_tensor(
                t1[:, 0, 0:H], w1c0r, af[:, 0:1], t0[:, 0, :], OP.mult, OP.add
            )
            dve.scalar_tensor_tensor(
                t1[:, 1, 0:H], w1c0r, af[:, 1:2], t0[:, 1, :], OP.mult, OP.add
            )
            # two extra hidden units ride along: u[32] = +1e30*empty (W2
            # coefficient -1 -> the empty-graph penalty) and u[33] = 1
            # (W2 coefficient b2 -> the bias); both survive the relu
            dve.tensor_copy(t1[:, 0, H : H + 1], pen[:, 0:1])
            dve.tensor_copy(t1[:, 1, H : H + 1], pen[:, 1:2])
            D()
            # z = sum_j relu(u_j) * w2ext_j  (w2ext = [W2, -1, b2])
            w2e = sb_blob[:, q + 4 * H : q + 5 * H + 2]
            dve.scalar_tensor_tensor(
                hws[:, 0, :], t1[:, 0, :], 0.0, w2e, OP.max, OP.mult,
                accum_out=z[:, 0:1],
            )
            dve.scalar_tensor_tensor(
                hws[:, 1, :], t1[:, 1, :], 0.0, w2e, OP.max, OP.mult,
                accum_out=z[:, 1:2],
            ).then_inc(dve_sem, 1)

    return nc


def kernel(
    node_features,
    batch,
    graph_embedding=None,
    W1=None,
    b1=None,
    W2=None,
    b2=None,
    num_graphs=None,
    **_unused,
):
    global LAST_EXEC_TIME_NS, LAST_PROFILE

    node_features = np.asarray(node_features)
    batch = np.asarray(batch)
    N = batch.shape[0]

    # CSR-style boundaries of the sorted index (pure index prep, O(G log N))
    bounds = np.searchsorted(batch, np.arange(G + 1, dtype=batch.dtype))
    counts = np.diff(bounds)
    # pair the largest-count graph with the smallest etc. so the padded
    # row length R tracks 2*mean instead of the worst adjacent pair
    order = np.argsort(counts, kind="stable")
    ga = order[: G // 2]  # slot-0 graph of pair j
    gb = order[G // 2 :][::-1]  # slot-1 graph of pair j
    R = int(np.ceil((counts[ga] + counts[gb]).max() / 8.0)) * 8

    # signed values: slot-1 nodes negated; exact zeros nudged so the
    # device can recover counts from the sign (values are >= 0 here)
    slot_of_graph = np.empty(G, np.int8)
    slot_of_graph[ga] = 0
    slot_of_graph[gb] = 1
    sgn = 1.0 - 2.0 * slot_of_graph[batch].astype(np.float32)
    colsT = np.empty((2, N), np.float32)

    # strided 512MB read; numpy slice copies release the GIL, so chunked
    # threads overlap the memory stalls
    def _extract(lo, hi):
        colsT[0, lo:hi] = node_features[lo:hi, MODIFIER_COL]
        colsT[1, lo:hi] = node_features[lo:hi, OWNER_COL]

    nchunk = 8
    bounds_t = [N * i // nchunk for i in range(nchunk + 1)]
    with ThreadPoolExecutor(nchunk) as ex:
        list(ex.map(lambda i: _extract(bounds_t[i], bounds_t[i + 1]), range(nchunk)))
    assert colsT[0].min() >= 0.0, "sign encoding needs non-negative col values"
    zz = colsT[0] == 0.0
    if zz.any():
        colsT[0, zz] = 1e-30
    vsT = colsT * sgn

    W1 = np.asarray(W1, np.float32)  # [32, 2]
    W2 = np.asarray(W2, np.float32)  # [1, 32]
    b1 = np.asarray(b1, np.float32)
    b2v = np.float32(np.asarray(b2, np.float32).reshape(()))

    blob_all = np.zeros((N_CORES, P, 2 * R + NPAR), np.float32)
    q = 2 * R
    blob_all[:, :, q + 0 * H : q + 1 * H] = W1[:, 0]
    blob_all[:, :, q + 1 * H : q + 2 * H] = W1[:, 1]
    blob_all[:, :, q + 2 * H : q + 3 * H] = -W1[:, 1]
    blob_all[:, :, q + 3 * H : q + 4 * H] = b1
    blob_all[:, :, q + 4 * H : q + 5 * H] = W2[0]
    blob_all[:, :, q + 5 * H] = -1.0
    blob_all[:, :, q + 5 * H + 1] = b2v
    flat = blob_all.reshape(N_CORES * P, 2 * R + NPAR)
    for j in range(N_CORES * P):
        o = 0
        for g in (ga[j], gb[j]):
            s, t = bounds[g], bounds[g + 1]
            if t > s:
                L = t - s
                flat[j, o : o + L] = vsT[0, s:t]
                flat[j, R + o : R + o + L] = vsT[1, s:t]
                o += L

    in_maps = [{"blob": blob_all[c]} for c in range(N_CORES)]

    global LAST_R
    LAST_R = R
    nc = _build_bass(R)
    trace = os.environ.get("BASS_KERNEL_PROFILE") == "1"
    res = run_bass_kernel_spmd(nc, in_maps, list(range(N_CORES)), trace=trace)
    LAST_EXEC_TIME_NS = getattr(res, "exec_time_ns", None)
    LAST_PROFILE = getattr(res, "profile_json", None)

    # pair j = (core j//P, partition j%P); slot 0 -> graph ga[j], slot 1 -> gb[j]
    allres = np.concatenate(
        [res.results[c]["score_out"] for c in range(N_CORES)], axis=0
    )  # [1024, 2]
    scores = np.empty((G,), np.float32)
    scores[ga] = allres[:, 0]
    scores[gb] = allres[:, 1]
    return scores
